# revision 25
# baseline (speedup 1.0000x reference)
"""Trainium2 Bass kernel for nn_Attention_91293824844283.

Multi-head attention (identity rep): per-head 1x1-conv Q/K/V projections,
softmax(Q K^T / sqrt(E)) V, per-head output projection summed over heads.

Shapes: B=4, N=2048, D=512, H=8, E=64.

Sharding over 8 cores: core c -> (batch b = c//2, head-group g = c%2 of 4
heads). Each core computes the partial output sum over its 4 heads for its
batch; host adds the two partials per batch.

Device-side layout/algorithm (per core):
  - Host supplies x2[b].T, x1[b].T, v[b].T (bf16) plus packed transposed
    weights. Scale 1/sqrt(E) is folded into Wq.
  - Q^T/K^T [E,N] computed per head-pair (2x64 rows packed into 128
    partitions).
  - V [N, 4*66] with a ones column per head (slot width 66) so the PV
    matmul also produces the softmax denominators (M=65).
  - Attention runs per (pair, nq-quarter): the two heads' S^T matmuls
    (K=64) land in disjoint PE row groups (partitions 0:64 / 64:128) and
    run concurrently; both heads' scores share one [128,1024] PSUM tile
    so a single ACT exp op serves the pair. PV accumulates each head's
    rep~^T [65, 512] in PSUM over the 16 nk tiles.
  - rep~^T -> SBUF, PE-transpose 128-col chunks, reciprocal of the sums
    column, per-partition scale (DVE), PE-transpose back to rep^T (bf16).
  - Pair-1 Q/K projections are emitted after pair-0 attention so the
    scheduler uses them as PE gap filler (2 spare PSUM banks).
  - Output projection: out[nq,D] += rep^T.T @ Wo^T accumulated over the 4
    heads in PSUM, staged to SBUF, DMA'd to DRAM fp32.
"""

import numpy as np
import ml_dtypes
from contextlib import ExitStack

B, N, D, H, E = 4, 2048, 512, 8, 64
HPC = 4            # heads per core
N_CORES = 8
NKT = N // 128     # 16 nk tiles
VSLOT = 66         # V slot: 64 V cols + 1 ones col + 1 pad
KT = D // 128      # 4 contraction tiles for projections
QW = 512           # nq quarter width

_CACHE = {}


def _build():
    import concourse.tile as tile
    from concourse import bacc, mybir

    bf16 = mybir.dt.bfloat16
    f32 = mybir.dt.float32
    Exp = mybir.ActivationFunctionType.Exp

    nc = bacc.Bacc(
        "TRN2", target_bir_lowering=False, debug=False, num_devices=N_CORES
    )
    xqT = nc.dram_tensor("xqT", [KT, 128, N], bf16, kind="ExternalInput").ap()
    xkT = nc.dram_tensor("xkT", [KT, 128, N], bf16, kind="ExternalInput").ap()
    vT = nc.dram_tensor("vT", [KT, 128, N], bf16, kind="ExternalInput").ap()
    wqT = nc.dram_tensor("wqT", [2, KT, 128, 128], bf16, kind="ExternalInput").ap()
    wkT = nc.dram_tensor("wkT", [2, KT, 128, 128], bf16, kind="ExternalInput").ap()
    wvT = nc.dram_tensor("wvT", [KT, 128, HPC * E], bf16, kind="ExternalInput").ap()
    woT = nc.dram_tensor("woT", [HPC, E, D], bf16, kind="ExternalInput").ap()
    identf = nc.dram_tensor("identf", [128, 128], f32, kind="ExternalInput").ap()
    identb = nc.dram_tensor("identb", [128, 128], bf16, kind="ExternalInput").ap()
    outp = nc.dram_tensor("outp", [NKT, 128, D], f32, kind="ExternalOutput").ap()

    with tile.TileContext(nc) as tc, ExitStack() as ctx:
        cp = ctx.enter_context(tc.tile_pool(name="const", bufs=1))

        # --- persistent SBUF tiles ---
        xq = [cp.tile([128, N], bf16, tag=f"xq{k}", name=f"xq{k}") for k in range(KT)]
        xk = [cp.tile([128, N], bf16, tag=f"xk{k}", name=f"xk{k}") for k in range(KT)]
        xv = [cp.tile([128, N], bf16, tag=f"xv{k}", name=f"xv{k}") for k in range(KT)]
        wq = [[cp.tile([128, 128], bf16, tag=f"wq{p}{k}", name=f"wq{p}{k}")
               for k in range(KT)] for p in range(2)]
        wk = [[cp.tile([128, 128], bf16, tag=f"wk{p}{k}", name=f"wk{p}{k}")
               for k in range(KT)] for p in range(2)]
        wv = [cp.tile([128, HPC * E], bf16, tag=f"wv{k}", name=f"wv{k}")
              for k in range(KT)]
        wo = [cp.tile([E, D], bf16, tag=f"wo{h}", name=f"wo{h}") for h in range(HPC)]
        idf = cp.tile([128, 128], f32, tag="idf")
        qt = [cp.tile([128, N], bf16, tag=f"qt{p}", name=f"qt{p}") for p in range(2)]
        kt = [cp.tile([128, N], bf16, tag=f"kt{p}", name=f"kt{p}") for p in range(2)]
        vaug = [cp.tile([128, HPC * VSLOT], bf16, tag=f"va{t}", name=f"va{t}")
                for t in range(NKT)]
        repbf16 = [cp.tile([E, N], bf16, tag=f"rb{h}", name=f"rb{h}")
                   for h in range(HPC)]
        idb = cp.tile([128, 128], bf16, tag="idb")

        # --- input DMAs, chunked 512 columns at a time so the first
        # projection matmuls can start after ~1/4 of the data has landed.
        # K path first (attention quarter 0 sweeps all of K but needs only
        # the first Q quarter), then V, then the remaining Q quarters.
        de = [nc.sync, nc.scalar]   # both HWDGE-capable queue sets
        for k in range(KT):
            for p in range(2):
                de[k % 2].dma_start(wq[p][k][:], wqT[p, k])
                de[(k + 1) % 2].dma_start(wk[p][k][:], wkT[p, k])
            de[k % 2].dma_start(wv[k][:], wvT[k])
        for c in range(4):
            sl = slice(c * 512, (c + 1) * 512)
            for k in range(KT):
                de[k % 2].dma_start(xk[k][:, sl], xkT[k][:, sl])
            if c == 0:
                for k in range(KT):
                    de[(k + 1) % 2].dma_start(xq[k][:, sl], xqT[k][:, sl])
        for c in range(4):
            sl = slice(c * 512, (c + 1) * 512)
            for k in range(KT):
                de[k % 2].dma_start(xv[k][:, sl], vT[k][:, sl])
            if c >= 1:
                qsl_ = slice((c - 1) * 512, c * 512)
                for k in range(KT):
                    de[(k + 1) % 2].dma_start(xq[k][:, qsl_], xqT[k][:, qsl_])
        sl = slice(3 * 512, 4 * 512)
        for k in range(KT):
            de[(k + 1) % 2].dma_start(xq[k][:, sl], xqT[k][:, sl])
        for h in range(HPC):
            nc.sync.dma_start(wo[h][:], woT[h])
        nc.sync.dma_start(idf[:], identf[:])
        nc.sync.dma_start(idb[:], identb[:])

        # --- PE warmup burst: dependency-free dummy matmuls fill the DMA
        # window and push HAM to K=8/8 before the first projection.
        warm_sb = cp.tile([128, 512], bf16, tag="warm_sb")
        nc.gpsimd.memset(warm_sb[:], 0.0)
        with tc.tile_pool(name="warmps", bufs=1, space="PSUM") as wps:
            wpt = wps.tile([128, 512], f32, tag="w", name="warm_ps")
            for i in range(32):
                nc.tensor.matmul(wpt[:], warm_sb[:, 0:128], warm_sb[:],
                                 start=True, stop=True)

        def proj_chunk(pool, dst, w, x, c, tag="proj"):
            ps = pool.tile([128, 512], f32, tag=tag, name="proj_ps")
            sl = slice(c * 512, (c + 1) * 512)
            for k in range(KT):
                nc.tensor.matmul(
                    ps[:], w[k][:], x[k][:, sl],
                    start=(k == 0), stop=(k == KT - 1),
                )
            nc.vector.tensor_copy(dst[:, sl], ps[:])

        def qk_proj(pool, p, tag="proj"):
            for c in range(4):
                proj_chunk(pool, kt[p], wk[p], xk, c, tag)
            for c in range(4):
                proj_chunk(pool, qt[p], wq[p], xq, c, tag)

        # --- attention pools (created before projections: a closing
        # projection pool would barrier attention PSUM allocation behind
        # ALL upfront work; instead projections share the fill pool).
        # PSUM: s pair tile 2 banks x bufs=2 + rep 2x1 bank + fill 2x1 = 8.
        sp = ctx.enter_context(tc.tile_pool(name="spsum", bufs=2, space="PSUM"))
        rp = ctx.enter_context(tc.tile_pool(name="rpsum", bufs=1, space="PSUM"))
        fpp = ctx.enter_context(tc.tile_pool(name="fill", bufs=2, space="PSUM"))
        ptp = ctx.enter_context(tc.tile_pool(name="ptile", bufs=4))
        smp = ctx.enter_context(tc.tile_pool(name="small", bufs=6))

        # --- upfront projections, ordered to unblock attention quarter 0:
        # K pair-0 (all chunks) + Q pair-0 chunk 0, then V, then Q rest.
        for c in range(4):
            proj_chunk(fpp, kt[0], wk[0], xk, c, tag="f")
        proj_chunk(fpp, qt[0], wq[0], xq, 0, tag="f")

        def vproj_t(t):
            # build vaug[t] just-in-time inside attention quarter 0
            nc.gpsimd.memset(vaug[t][:], 1.0)
            ps = fpp.tile([128, HPC * E], f32, tag="f", name="vproj_ps")
            tsl = slice(t * 128, (t + 1) * 128)
            for k in range(KT):
                nc.tensor.matmul(
                    ps[:], xv[k][:, tsl], wv[k][:],
                    start=(k == 0), stop=(k == KT - 1),
                )
            for h in range(HPC):
                nc.vector.tensor_copy(
                    vaug[t][:, h * VSLOT:h * VSLOT + E],
                    ps[:, h * E:(h + 1) * E],
                )

        def attention_pair(p, after_quarter=None):
            for q4 in range(4):
                qoff = q4 * QW
                rep = [
                    rp.tile([65, QW], f32, tag=f"rep{s}", name=f"rep{s}")
                    for s in range(2)
                ]
                for t in range(NKT):
                    tsl = slice(t * 128, (t + 1) * 128)
                    spair = sp.tile([128, 2 * QW], f32, tag="s", name="spair")
                    for s in range(2):
                        esl = slice(s * 64, (s + 1) * 64)
                        nc.tensor.matmul(
                            spair[:, s * QW:(s + 1) * QW],
                            kt[p][esl, tsl], qt[p][esl, qoff:qoff + QW],
                            start=True, stop=True,
                        )
                    pt = ptp.tile([128, 2 * QW], bf16, tag="p", name="pt")
                    nc.scalar.activation(pt[:], spair[:], Exp)
                    if p == 0 and q4 == 0:
                        vproj_t(t)
                    for s in range(2):
                        h = 2 * p + s
                        vsl = slice(h * VSLOT, h * VSLOT + 65)
                        nc.tensor.matmul(
                            rep[s][:],
                            vaug[t][:, vsl], pt[:, s * QW:(s + 1) * QW],
                            start=(t == 0), stop=(t == NKT - 1),
                        )
                # drain + normalize (transpose, scale rows, transpose back)
                for s in range(2):
                    h = 2 * p + s
                    rts = smp.tile([65, QW], f32, tag=f"rts{s}", name=f"rts{s}")
                    nc.vector.tensor_copy(rts[:], rep[s][:])
                    for tt in range(QW // 128):
                        csl = slice(tt * 128, (tt + 1) * 128)
                        osl = slice(qoff + tt * 128, qoff + (tt + 1) * 128)
                        tr1 = fpp.tile([128, 65], f32, tag="f", name=f"tr1_{s}")
                        nc.tensor.transpose(tr1[:], rts[:, csl], idf[0:65, 0:65])
                        r = smp.tile([128, 1], f32, tag="r")
                        nc.vector.reciprocal(r[:], tr1[:, 64:65])
                        rb = smp.tile([128, E], bf16, tag="rb")
                        nc.vector.tensor_scalar_mul(rb[:], tr1[:, 0:E], r[:])
                        tr2 = fpp.tile([E, 128], bf16, tag="f", name=f"tr2_{s}")
                        nc.tensor.transpose(tr2[:], rb[:], idb[:])
                        nc.vector.tensor_copy(repbf16[h][:, osl], tr2[:])
                if after_quarter is not None:
                    after_quarter(q4)

        def outproj_quarter(q4):
            # out tiles for nq quarter q4 (all 4 heads' rep ready by now)
            for t in range(4 * q4, 4 * q4 + 4):
                tsl = slice(t * 128, (t + 1) * 128)
                ops = fpp.tile([128, D], f32, tag="f", name="ops")
                for h in range(HPC):
                    nc.tensor.matmul(
                        ops[:], repbf16[h][:, tsl], wo[h][:],
                        start=(h == 0), stop=(h == HPC - 1),
                    )
                ost = ptp.tile([128, D], f32, tag="ost")
                nc.vector.tensor_copy(ost[:], ops[:])
                nc.sync.dma_start(outp[t], ost[:])

        # pair-1 projections spread across pair-0's quarters: each
        # quarter's tail emits two chunks, so the scheduler drains them in
        # PE gaps well before pair-1 attention needs them.
        def pair1_proj_part(q4):
            if q4 < 2:
                proj_chunk(fpp, kt[1], wk[1], xk, 2 * q4, tag="f")
                proj_chunk(fpp, kt[1], wk[1], xk, 2 * q4 + 1, tag="f")
            else:
                proj_chunk(fpp, qt[1], wq[1], xq, 2 * (q4 - 2), tag="f")
                proj_chunk(fpp, qt[1], wq[1], xq, 2 * (q4 - 2) + 1, tag="f")

        def pair0_tail(q4):
            if q4 < 3:
                proj_chunk(fpp, qt[0], wq[0], xq, q4 + 1, tag="f")
            pair1_proj_part(q4)

        attention_pair(0, after_quarter=pair0_tail)
        attention_pair(1, after_quarter=outproj_quarter)

    nc.compile()
    return nc


def _prep_core_inputs(c, x1, x2, v, Wq, Wk, Wv, Wo, identf, identb):
    bf = ml_dtypes.bfloat16
    b, g = c // 2, c % 2
    hs = slice(g * HPC, (g + 1) * HPC)
    wq = (Wq[hs] * (1.0 / np.sqrt(E))).astype(np.float32)   # fold 1/sqrt(E)
    wk, wv, wo = Wk[hs], Wv[hs], Wo[hs]

    def t_pack_pair(w):
        # [4,E,D] -> per pair p: concat(w[2p].T, w[2p+1].T, axis=1) [D,128]
        out = np.empty((2, KT, 128, 128), bf)
        for p in range(2):
            m = np.concatenate([w[2 * p].T, w[2 * p + 1].T], axis=1)  # [D,128]
            out[p] = m.reshape(KT, 128, 128).astype(bf)
        return out

    xq = np.ascontiguousarray(x2[b].T).astype(bf).reshape(KT, 128, N)
    xk = np.ascontiguousarray(x1[b].T).astype(bf).reshape(KT, 128, N)
    xv = np.ascontiguousarray(v[b].T).astype(bf).reshape(KT, 128, N)
    wvT = np.concatenate([wv[h].T for h in range(HPC)], axis=1)  # [D, 256]
    woT = np.stack([wo[h].T for h in range(HPC)])                # [4, E, D]
    return {
        "xqT": xq, "xkT": xk, "vT": xv,
        "wqT": t_pack_pair(wq), "wkT": t_pack_pair(wk),
        "wvT": np.ascontiguousarray(wvT).astype(bf).reshape(KT, 128, HPC * E),
        "woT": woT.astype(bf),
        "identf": identf, "identb": identb,
    }


def kernel(**inputs):
    from concourse.bass_utils import run_bass_kernel_spmd

    x1 = np.asarray(inputs["x1"], np.float32)
    x2 = np.asarray(inputs["x2"], np.float32)
    v = np.asarray(inputs["v"], np.float32)
    Wq = np.asarray(inputs["Wq"], np.float32)
    Wk = np.asarray(inputs["Wk"], np.float32)
    Wv = np.asarray(inputs["Wv"], np.float32)
    Wo = np.asarray(inputs["Wo"], np.float32)

    if "nc" not in _CACHE:
        _CACHE["nc"] = _build()
    nc = _CACHE["nc"]

    identf = np.eye(128, dtype=np.float32)
    identb = np.eye(128, dtype=ml_dtypes.bfloat16)
    in_maps = [
        _prep_core_inputs(c, x1, x2, v, Wq, Wk, Wv, Wo, identf, identb)
        for c in range(N_CORES)
    ]
    res = run_bass_kernel_spmd(nc, in_maps, list(range(N_CORES)))
    out = np.empty((B, N, D), np.float32)
    for b in range(B):
        out[b] = (
            res.results[2 * b]["outp"].reshape(N, D)
            + res.results[2 * b + 1]["outp"].reshape(N, D)
        )
    return out


# revision 26
# speedup vs baseline: 1.0930x; 1.0930x over previous
"""Trainium2 Bass kernel for nn_Attention_91293824844283.

Multi-head attention (identity rep): per-head 1x1-conv Q/K/V projections,
softmax(Q K^T / sqrt(E)) V, per-head output projection summed over heads.

Shapes: B=4, N=2048, D=512, H=8, E=64.

Sharding over 8 cores: core c -> (batch b = c//2, head-group g = c%2 of 4
heads). Each core computes the partial output sum over its 4 heads for its
batch; host adds the two partials per batch.

Device-side layout/algorithm (per core):
  - Host supplies x2[b].T, x1[b].T, v[b].T (bf16) plus packed transposed
    weights. Scale 1/sqrt(E) is folded into Wq.
  - Q^T/K^T [E,N] computed per head-pair (2x64 rows packed into 128
    partitions).
  - V [N, 4*66] with a ones column per head (slot width 66) so the PV
    matmul also produces the softmax denominators (M=65).
  - Attention runs per (pair, nq-quarter): the two heads' S^T matmuls
    (K=64) land in disjoint PE row groups (partitions 0:64 / 64:128) and
    run concurrently; both heads' scores share one [128,1024] PSUM tile
    so a single ACT exp op serves the pair. PV accumulates each head's
    rep~^T [65, 512] in PSUM over the 16 nk tiles.
  - rep~^T -> SBUF, PE-transpose 128-col chunks, reciprocal of the sums
    column, per-partition scale (DVE), PE-transpose back to rep^T (bf16).
  - Pair-1 Q/K projections are emitted after pair-0 attention so the
    scheduler uses them as PE gap filler (2 spare PSUM banks).
  - Output projection: out[nq,D] += rep^T.T @ Wo^T accumulated over the 4
    heads in PSUM, staged to SBUF, DMA'd to DRAM fp32.
"""

import numpy as np
import ml_dtypes
from contextlib import ExitStack

B, N, D, H, E = 4, 2048, 512, 8, 64
HPC = 4            # heads per core
N_CORES = 8
NKT = N // 128     # 16 nk tiles
VSLOT = 66         # V slot: 64 V cols + 1 ones col + 1 pad
KT = D // 128      # 4 contraction tiles for projections
QW = 512           # nq quarter width

_CACHE = {}


def _build():
    import concourse.tile as tile
    from concourse import bacc, mybir

    bf16 = mybir.dt.bfloat16
    f32 = mybir.dt.float32
    Exp = mybir.ActivationFunctionType.Exp

    nc = bacc.Bacc(
        "TRN2", target_bir_lowering=False, debug=False, num_devices=N_CORES
    )
    xqT = nc.dram_tensor("xqT", [KT, 128, N], bf16, kind="ExternalInput").ap()
    xkT = nc.dram_tensor("xkT", [KT, 128, N], bf16, kind="ExternalInput").ap()
    vT = nc.dram_tensor("vT", [KT, 128, N], bf16, kind="ExternalInput").ap()
    wqT = nc.dram_tensor("wqT", [2, KT, 128, 128], bf16, kind="ExternalInput").ap()
    wkT = nc.dram_tensor("wkT", [2, KT, 128, 128], bf16, kind="ExternalInput").ap()
    wvT = nc.dram_tensor("wvT", [KT, 128, HPC * E], bf16, kind="ExternalInput").ap()
    woT = nc.dram_tensor("woT", [HPC, E, D], bf16, kind="ExternalInput").ap()
    identf = nc.dram_tensor("identf", [128, 128], f32, kind="ExternalInput").ap()
    identb = nc.dram_tensor("identb", [128, 128], bf16, kind="ExternalInput").ap()
    outp = nc.dram_tensor("outp", [NKT, 128, D], f32, kind="ExternalOutput").ap()

    with tile.TileContext(nc) as tc, ExitStack() as ctx:
        cp = ctx.enter_context(tc.tile_pool(name="const", bufs=1))

        # --- persistent SBUF tiles ---
        xq = [cp.tile([128, N], bf16, tag=f"xq{k}", name=f"xq{k}") for k in range(KT)]
        xk = [cp.tile([128, N], bf16, tag=f"xk{k}", name=f"xk{k}") for k in range(KT)]
        xv = [cp.tile([128, N], bf16, tag=f"xv{k}", name=f"xv{k}") for k in range(KT)]
        wq = [[cp.tile([128, 128], bf16, tag=f"wq{p}{k}", name=f"wq{p}{k}")
               for k in range(KT)] for p in range(2)]
        wk = [[cp.tile([128, 128], bf16, tag=f"wk{p}{k}", name=f"wk{p}{k}")
               for k in range(KT)] for p in range(2)]
        wv = [cp.tile([128, HPC * E], bf16, tag=f"wv{k}", name=f"wv{k}")
              for k in range(KT)]
        wo = [cp.tile([E, D], bf16, tag=f"wo{h}", name=f"wo{h}") for h in range(HPC)]
        idf = cp.tile([128, 128], f32, tag="idf")
        qt = [cp.tile([128, N], bf16, tag=f"qt{p}", name=f"qt{p}") for p in range(2)]
        kt = [cp.tile([128, N], bf16, tag=f"kt{p}", name=f"kt{p}") for p in range(2)]
        vaug = [cp.tile([128, HPC * VSLOT], bf16, tag=f"va{t}", name=f"va{t}")
                for t in range(NKT)]
        repbf16 = [cp.tile([E, N], bf16, tag=f"rb{h}", name=f"rb{h}")
                   for h in range(HPC)]
        idb = cp.tile([128, 128], bf16, tag="idb")

        # --- input DMAs, chunked 512 columns at a time so the first
        # projection matmuls can start after ~1/4 of the data has landed.
        # K path first (attention quarter 0 sweeps all of K but needs only
        # the first Q quarter), then V, then the remaining Q quarters.
        de = [nc.sync, nc.scalar]   # both HWDGE-capable queue sets
        for k in range(KT):
            for p in range(2):
                de[k % 2].dma_start(wq[p][k][:], wqT[p, k])
                de[(k + 1) % 2].dma_start(wk[p][k][:], wkT[p, k])
            de[k % 2].dma_start(wv[k][:], wvT[k])
        for c in range(4):
            sl = slice(c * 512, (c + 1) * 512)
            for k in range(KT):
                de[k % 2].dma_start(xk[k][:, sl], xkT[k][:, sl])
            if c == 0:
                for k in range(KT):
                    de[(k + 1) % 2].dma_start(xq[k][:, sl], xqT[k][:, sl])
        for c in range(4):
            sl = slice(c * 512, (c + 1) * 512)
            for k in range(KT):
                de[k % 2].dma_start(xv[k][:, sl], vT[k][:, sl])
        for c in range(1, 4):
            sl = slice(c * 512, (c + 1) * 512)
            for k in range(KT):
                de[(k + 1) % 2].dma_start(xq[k][:, sl], xqT[k][:, sl])
        for h in range(HPC):
            nc.sync.dma_start(wo[h][:], woT[h])
        nc.sync.dma_start(idf[:], identf[:])
        nc.sync.dma_start(idb[:], identb[:])

        # --- PE warmup burst: dependency-free dummy matmuls fill the DMA
        # window and push HAM to K=8/8 before the first projection.
        warm_sb = cp.tile([128, 512], bf16, tag="warm_sb")
        nc.gpsimd.memset(warm_sb[:], 0.0)
        with tc.tile_pool(name="warmps", bufs=1, space="PSUM") as wps:
            wpt = wps.tile([128, 512], f32, tag="w", name="warm_ps")
            for i in range(32):
                nc.tensor.matmul(wpt[:], warm_sb[:, 0:128], warm_sb[:],
                                 start=True, stop=True)

        def proj_chunk(pool, dst, w, x, c, tag="proj"):
            ps = pool.tile([128, 512], f32, tag=tag, name="proj_ps")
            sl = slice(c * 512, (c + 1) * 512)
            for k in range(KT):
                nc.tensor.matmul(
                    ps[:], w[k][:], x[k][:, sl],
                    start=(k == 0), stop=(k == KT - 1),
                )
            nc.vector.tensor_copy(dst[:, sl], ps[:])

        def qk_proj(pool, p, tag="proj"):
            for c in range(4):
                proj_chunk(pool, kt[p], wk[p], xk, c, tag)
            for c in range(4):
                proj_chunk(pool, qt[p], wq[p], xq, c, tag)

        # --- attention pools (created before projections: a closing
        # projection pool would barrier attention PSUM allocation behind
        # ALL upfront work; instead projections share the fill pool).
        # PSUM: s pair tile 2 banks x bufs=2 + rep 2x1 bank + fill 2x1 = 8.
        sp = ctx.enter_context(tc.tile_pool(name="spsum", bufs=2, space="PSUM"))
        rp = ctx.enter_context(tc.tile_pool(name="rpsum", bufs=1, space="PSUM"))
        fpp = ctx.enter_context(tc.tile_pool(name="fill", bufs=2, space="PSUM"))
        ptp = ctx.enter_context(tc.tile_pool(name="ptile", bufs=4))
        smp = ctx.enter_context(tc.tile_pool(name="small", bufs=6))

        # --- upfront projections, ordered to unblock attention quarter 0:
        # K pair-0 (all chunks) + Q pair-0 chunk 0, then V, then Q rest.
        for c in range(4):
            proj_chunk(fpp, kt[0], wk[0], xk, c, tag="f")
        proj_chunk(fpp, qt[0], wq[0], xq, 0, tag="f")
        for t in range(NKT):
            nc.gpsimd.memset(vaug[t][:], 1.0)
            ps = fpp.tile([128, HPC * E], f32, tag="f", name="vproj_ps")
            tsl = slice(t * 128, (t + 1) * 128)
            for k in range(KT):
                nc.tensor.matmul(
                    ps[:], xv[k][:, tsl], wv[k][:],
                    start=(k == 0), stop=(k == KT - 1),
                )
            for h in range(HPC):
                nc.vector.tensor_copy(
                    vaug[t][:, h * VSLOT:h * VSLOT + E],
                    ps[:, h * E:(h + 1) * E],
                )
        for c in range(1, 4):
            proj_chunk(fpp, qt[0], wq[0], xq, c, tag="f")

        def attention_pair(p, after_quarter=None):
            for q4 in range(4):
                qoff = q4 * QW
                rep = [
                    rp.tile([65, QW], f32, tag=f"rep{s}", name=f"rep{s}")
                    for s in range(2)
                ]
                for t in range(NKT):
                    tsl = slice(t * 128, (t + 1) * 128)
                    spair = sp.tile([128, 2 * QW], f32, tag="s", name="spair")
                    for s in range(2):
                        esl = slice(s * 64, (s + 1) * 64)
                        nc.tensor.matmul(
                            spair[:, s * QW:(s + 1) * QW],
                            kt[p][esl, tsl], qt[p][esl, qoff:qoff + QW],
                            start=True, stop=True,
                        )
                    pt = ptp.tile([128, 2 * QW], bf16, tag="p", name="pt")
                    nc.scalar.activation(pt[:], spair[:], Exp)
                    for s in range(2):
                        h = 2 * p + s
                        vsl = slice(h * VSLOT, h * VSLOT + 65)
                        nc.tensor.matmul(
                            rep[s][:],
                            vaug[t][:, vsl], pt[:, s * QW:(s + 1) * QW],
                            start=(t == 0), stop=(t == NKT - 1),
                        )
                # drain + normalize (transpose, scale rows, transpose back)
                for s in range(2):
                    h = 2 * p + s
                    rts = smp.tile([65, QW], f32, tag=f"rts{s}", name=f"rts{s}")
                    nc.vector.tensor_copy(rts[:], rep[s][:])
                    for tt in range(QW // 128):
                        csl = slice(tt * 128, (tt + 1) * 128)
                        osl = slice(qoff + tt * 128, qoff + (tt + 1) * 128)
                        tr1 = fpp.tile([128, 65], f32, tag="f", name=f"tr1_{s}")
                        nc.tensor.transpose(tr1[:], rts[:, csl], idf[0:65, 0:65])
                        r = smp.tile([128, 1], f32, tag="r")
                        nc.vector.reciprocal(r[:], tr1[:, 64:65])
                        rb = smp.tile([128, E], bf16, tag="rb")
                        nc.vector.tensor_scalar_mul(rb[:], tr1[:, 0:E], r[:])
                        tr2 = fpp.tile([E, 128], bf16, tag="f", name=f"tr2_{s}")
                        nc.tensor.transpose(tr2[:], rb[:], idb[:])
                        nc.vector.tensor_copy(repbf16[h][:, osl], tr2[:])
                if after_quarter is not None:
                    after_quarter(q4)

        def outproj_quarter(q4):
            # out tiles for nq quarter q4 (all 4 heads' rep ready by now)
            for t in range(4 * q4, 4 * q4 + 4):
                tsl = slice(t * 128, (t + 1) * 128)
                ops = fpp.tile([128, D], f32, tag="f", name="ops")
                for h in range(HPC):
                    nc.tensor.matmul(
                        ops[:], repbf16[h][:, tsl], wo[h][:],
                        start=(h == 0), stop=(h == HPC - 1),
                    )
                ost = ptp.tile([128, D], f32, tag="ost")
                nc.vector.tensor_copy(ost[:], ops[:])
                nc.sync.dma_start(outp[t], ost[:])

        # pair-1 projections spread across pair-0's quarters: each
        # quarter's tail emits two chunks, so the scheduler drains them in
        # PE gaps well before pair-1 attention needs them.
        def pair1_proj_part(q4):
            if q4 < 2:
                proj_chunk(fpp, kt[1], wk[1], xk, 2 * q4, tag="f")
                proj_chunk(fpp, kt[1], wk[1], xk, 2 * q4 + 1, tag="f")
            else:
                proj_chunk(fpp, qt[1], wq[1], xq, 2 * (q4 - 2), tag="f")
                proj_chunk(fpp, qt[1], wq[1], xq, 2 * (q4 - 2) + 1, tag="f")

        attention_pair(0, after_quarter=pair1_proj_part)
        attention_pair(1, after_quarter=outproj_quarter)

    nc.compile()
    return nc


def _prep_core_inputs(c, x1, x2, v, Wq, Wk, Wv, Wo, identf, identb):
    bf = ml_dtypes.bfloat16
    b, g = c // 2, c % 2
    hs = slice(g * HPC, (g + 1) * HPC)
    wq = (Wq[hs] * (1.0 / np.sqrt(E))).astype(np.float32)   # fold 1/sqrt(E)
    wk, wv, wo = Wk[hs], Wv[hs], Wo[hs]

    def t_pack_pair(w):
        # [4,E,D] -> per pair p: concat(w[2p].T, w[2p+1].T, axis=1) [D,128]
        out = np.empty((2, KT, 128, 128), bf)
        for p in range(2):
            m = np.concatenate([w[2 * p].T, w[2 * p + 1].T], axis=1)  # [D,128]
            out[p] = m.reshape(KT, 128, 128).astype(bf)
        return out

    xq = np.ascontiguousarray(x2[b].T).astype(bf).reshape(KT, 128, N)
    xk = np.ascontiguousarray(x1[b].T).astype(bf).reshape(KT, 128, N)
    xv = np.ascontiguousarray(v[b].T).astype(bf).reshape(KT, 128, N)
    wvT = np.concatenate([wv[h].T for h in range(HPC)], axis=1)  # [D, 256]
    woT = np.stack([wo[h].T for h in range(HPC)])                # [4, E, D]
    return {
        "xqT": xq, "xkT": xk, "vT": xv,
        "wqT": t_pack_pair(wq), "wkT": t_pack_pair(wk),
        "wvT": np.ascontiguousarray(wvT).astype(bf).reshape(KT, 128, HPC * E),
        "woT": woT.astype(bf),
        "identf": identf, "identb": identb,
    }


def kernel(**inputs):
    from concourse.bass_utils import run_bass_kernel_spmd

    x1 = np.asarray(inputs["x1"], np.float32)
    x2 = np.asarray(inputs["x2"], np.float32)
    v = np.asarray(inputs["v"], np.float32)
    Wq = np.asarray(inputs["Wq"], np.float32)
    Wk = np.asarray(inputs["Wk"], np.float32)
    Wv = np.asarray(inputs["Wv"], np.float32)
    Wo = np.asarray(inputs["Wo"], np.float32)

    if "nc" not in _CACHE:
        _CACHE["nc"] = _build()
    nc = _CACHE["nc"]

    identf = np.eye(128, dtype=np.float32)
    identb = np.eye(128, dtype=ml_dtypes.bfloat16)
    in_maps = [
        _prep_core_inputs(c, x1, x2, v, Wq, Wk, Wv, Wo, identf, identb)
        for c in range(N_CORES)
    ]
    res = run_bass_kernel_spmd(nc, in_maps, list(range(N_CORES)))
    out = np.empty((B, N, D), np.float32)
    for b in range(B):
        out[b] = (
            res.results[2 * b]["outp"].reshape(N, D)
            + res.results[2 * b + 1]["outp"].reshape(N, D)
        )
    return out
